# revision 40
# baseline (speedup 1.0000x reference)
"""DSNT distance double loss on 8 Trainium2 cores.

Strategy: data-parallel over batch. Each of the 8 cores gets 4 batches
(= 8 maps of 512x512, one per (b, c)). On-device, per map:
  - ACT computes exp(x) (softmax without max-subtraction; inputs are
    standard normal so exp is safely in range),
  - PE contracts over partitions with a [ones; y-grid] stationary matrix
    to get column sums and y-weighted column sums in PSUM [2, 512],
  - DVE max/max_index give the per-partition top-1 value and its index
    for the target map (argmax side).
Host side: per map, reduce the 512 column sums to E/Ex/Ey (float64
dots), pick the argmax partition out of 128 candidates, decode
coordinates, and do the final O(B*C) loss math.
"""

import numpy as np

N_CORES = 8
B, C, H, W = 32, 2, 512, 512
BPC = B // N_CORES          # batches per core
MAPS = BPC * C              # maps per core
P = 128                     # SBUF partitions
F = (H * W) // P            # 2048 free elements per partition
NB = F // W                 # 4 column blocks of width 512

_CACHE = {}
TRACE = False
LAST_RESULTS = None


def _build(loop_reps=None, t_ring="sp", x_ring="sp", x_chunks=1,
           out_ring="gpsimd", t0_split=1, pair=False, io_bufs=3,
           tgt_bufs=6, psum_bufs=2, last_x_chunks=4):
    import concourse.bacc as bacc
    import concourse.mybir as mybir
    import concourse.tile as tile

    f32 = mybir.dt.float32
    bf16 = mybir.dt.bfloat16
    u32 = mybir.dt.uint32

    nc = bacc.Bacc("TRN2", target_bir_lowering=False, debug=False,
                   num_devices=N_CORES)

    inp = nc.dram_tensor("input", [MAPS, P, F], f32, kind="ExternalInput")
    tgt = nc.dram_tensor("target", [MAPS, P, F], f32, kind="ExternalInput")
    lhsw = nc.dram_tensor("lhsw", [P, 3 * NB], bf16, kind="ExternalInput")
    outS = nc.dram_tensor("outS", [3, MAPS * W], f32, kind="ExternalOutput")
    outM = nc.dram_tensor("outM", [P, 8 * MAPS], f32, kind="ExternalOutput")
    outI = nc.dram_tensor("outI", [P, 8 * MAPS], u32, kind="ExternalOutput")

    with tile.TileContext(nc) as tc:
        with (
            tc.tile_pool(name="io", bufs=io_bufs) as io_pool,
            tc.tile_pool(name="tgt", bufs=tgt_bufs) as tgt_pool,
            tc.tile_pool(name="const", bufs=1) as const_pool,
            tc.tile_pool(name="stage", bufs=1) as stage_pool,
            tc.tile_pool(name="psum", bufs=psum_bufs, space="PSUM") as psum_pool,
        ):
            lhs_t = const_pool.tile([P, 3 * NB], bf16)
            nc.scalar.dma_start(lhs_t[:], lhsw[:])

            mx_all = stage_pool.tile([P, 8 * MAPS], f32)
            ix_all = stage_pool.tile([P, 8 * MAPS], u32)
            stageS = stage_pool.tile([3, MAPS * W], f32)

            rings = {"sp": nc.sync, "act": nc.scalar, "gpsimd": nc.gpsimd}
            t_eng = rings[t_ring]
            x_eng = rings[x_ring]
            o_eng = rings[out_ring]

            def body_pair(_iv=None):
                # 2MB paired loads: halves DMA count and per-DMA overhead
                for j in range(MAPS // 2):
                    t_t = tgt_pool.tile([P, 2 * F], f32, tag="t")
                    t_eng.dma_start(
                        t_t[:].rearrange("p (k f) -> p k f", k=2),
                        tgt[2 * j:2 * j + 2].rearrange("k p f -> p k f"))
                    x_t = io_pool.tile([P, 2 * F], f32, tag="x")
                    x_eng.dma_start(
                        x_t[:].rearrange("p (k f) -> p k f", k=2),
                        inp[2 * j:2 * j + 2].rearrange("k p f -> p k f"))
                    e_t = io_pool.tile([P, 2 * F], bf16, tag="e")
                    nc.scalar.activation(e_t[:], x_t[:],
                                         mybir.ActivationFunctionType.Exp)
                    for m in range(2):
                        i = 2 * j + m
                        nc.vector.max(mx_all[:, 8 * i:8 * i + 8],
                                      t_t[:, m * F:(m + 1) * F])
                        nc.vector.max_index(ix_all[:, 8 * i:8 * i + 8],
                                            mx_all[:, 8 * i:8 * i + 8],
                                            t_t[:, m * F:(m + 1) * F])
                        ps = psum_pool.tile([3, W], f32)
                        for q in range(NB):
                            nc.tensor.matmul(
                                ps[:],
                                lhs_t[:, 3 * q:3 * q + 3],
                                e_t[:, m * F + W * q:m * F + W * (q + 1)],
                                start=(q == 0),
                                stop=(q == NB - 1),
                            )
                        nc.scalar.activation(
                            stageS[:, i * W:(i + 1) * W], ps[:],
                            mybir.ActivationFunctionType.Copy)

                o_eng.dma_start(outM[:], mx_all[:])
                o_eng.dma_start(outI[:], ix_all[:])
                nc.sync.dma_start(outS[:], stageS[:])

            def body(_iv=None):
                if pair:
                    return body_pair(_iv)
                for i in range(MAPS):
                    # target side: DVE max/argmax
                    t_t = tgt_pool.tile([P, F], f32, tag="t")
                    if i == 0 and t0_split > 1:
                        # split the first load so the DMA stream starts
                        # before descriptor generation of a full map ends
                        QF = F // t0_split
                        for h in range(t0_split):
                            cs = slice(h * QF, (h + 1) * QF)
                            t_eng.dma_start(t_t[:, cs], tgt[i][:, cs])
                    else:
                        t_eng.dma_start(t_t[:], tgt[i])
                    nc.vector.max(mx_all[:, 8 * i:8 * i + 8], t_t[:])
                    nc.vector.max_index(ix_all[:, 8 * i:8 * i + 8],
                                        mx_all[:, 8 * i:8 * i + 8], t_t[:])

                    # input side: chunked loads, ACT exp, PE sums
                    x_t = io_pool.tile([P, F], f32, tag="x")
                    e_t = io_pool.tile([P, F], bf16, tag="e")
                    ps = psum_pool.tile([3, W], f32)
                    nch = x_chunks
                    if last_x_chunks is not None and i == MAPS - 1:
                        nch = last_x_chunks
                    csz = F // nch
                    qpc = NB // nch
                    for h in range(nch):
                        cs = slice(h * csz, (h + 1) * csz)
                        x_eng.dma_start(x_t[:, cs], inp[i][:, cs])
                        nc.scalar.activation(e_t[:, cs], x_t[:, cs],
                                             mybir.ActivationFunctionType.Exp)
                        for q in range(h * qpc, (h + 1) * qpc):
                            nc.tensor.matmul(
                                ps[:],
                                lhs_t[:, 3 * q:3 * q + 3],
                                e_t[:, W * q:W * (q + 1)],
                                start=(q == 0),
                                stop=(q == NB - 1),
                            )
                    nc.scalar.activation(
                        stageS[:, i * W:(i + 1) * W], ps[:],
                        mybir.ActivationFunctionType.Copy)

                # parallel result-DMA generation: outM on SP, outI on Pool;
                # outS split so only the last map's 6KB slice trails the tail
                nc.sync.dma_start(outM[:], mx_all[:])
                o_eng.dma_start(outI[:], ix_all[:])
                cut = (MAPS - 1) * W
                nc.sync.dma_start(outS[:, :cut], stageS[:, :cut])
                nc.sync.dma_start(outS[:, cut:], stageS[:, cut:])

            if loop_reps is None:
                body()
            else:
                with tc.For_i(0, loop_reps, 1) as iv:
                    body(iv)

    nc.compile()
    return nc


def _consts():
    import ml_dtypes
    p = np.arange(P, dtype=np.float64)
    lhsw = np.zeros((P, 3 * NB), dtype=np.float64)
    for q in range(NB):
        yg = (NB * p + q + 1) / H
        yg_hi = yg.astype(ml_dtypes.bfloat16).astype(np.float64)
        lhsw[:, 3 * q] = 1.0
        lhsw[:, 3 * q + 1] = yg_hi
        lhsw[:, 3 * q + 2] = yg - yg_hi
    return lhsw.astype(ml_dtypes.bfloat16)


def kernel(input, target):
    global LAST_RESULTS
    from concourse.bass_utils import run_bass_kernel_spmd

    if "nc" not in _CACHE:
        _CACHE["nc"] = _build()
        _CACHE["lhsw"] = _consts()
    nc = _CACHE["nc"]
    lhsw = _CACHE["lhsw"]

    input = np.ascontiguousarray(np.asarray(input, dtype=np.float32))
    target = np.ascontiguousarray(np.asarray(target, dtype=np.float32))

    in_maps = []
    for s in range(N_CORES):
        sl = slice(s * BPC, (s + 1) * BPC)
        in_maps.append({
            "input": input[sl].reshape(MAPS, P, F),
            "target": target[sl].reshape(MAPS, P, F),
            "lhsw": lhsw,
        })

    res = run_bass_kernel_spmd(nc, in_maps, list(range(N_CORES)),
                               trace=TRACE)
    LAST_RESULTS = res

    # host finalize in float64
    xg = (np.arange(W, dtype=np.float64) + 1.0) / W
    px = np.zeros((B, C)); py = np.zeros((B, C))
    tx = np.zeros((B, C)); ty = np.zeros((B, C))
    for s in range(N_CORES):
        r = res.results[s]
        outS, outM, outI = r["outS"], r["outM"], r["outI"]
        for i in range(MAPS):
            b = s * BPC + i // C
            c = i % C
            colsum = outS[0, i * W:(i + 1) * W].astype(np.float64)
            ysum = (outS[1, i * W:(i + 1) * W].astype(np.float64)
                    + outS[2, i * W:(i + 1) * W].astype(np.float64))
            E = colsum.sum()
            px[b, c] = (colsum @ xg) / E
            py[b, c] = ysum.sum() / E
            mxcol = outM[:, 8 * i]
            k = int(np.argmax(mxcol))
            flat = k * F + int(outI[k, 8 * i])
            tx[b, c] = ((flat % W) + 1.0) / W
            ty[b, c] = ((flat // W) + 1.0) / H

    ed = np.sqrt((tx - px) ** 2 + (ty - py) ** 2)
    pd = np.sqrt((px[:, 0] - px[:, 1]) ** 2 + (py[:, 0] - py[:, 1]) ** 2)
    td = np.sqrt((tx[:, 0] - tx[:, 1]) ** 2 + (ty[:, 0] - ty[:, 1]) ** 2)
    s = ed.sum() + np.abs(pd - td).sum()
    return np.array([s / B], dtype=np.float32)
